# revision 1
# baseline (speedup 1.0000x reference)
"""HAttentionNetwork Trainium2 kernel.

Strategy (8 NeuronCores, data-parallel over bags):
- 4096 bags are split into 64 contiguous-bag chunks (8 per core), balanced by
  sentence count with <=64 bags per chunk (binary-search + greedy partition).
- Each chunk's sentences are padded to Tc tiles of 128 sentences; per-core
  arrays are "arranged" host-side so the device loop is fully static.
- Per 128-sentence tile, on device:
    FLT = C_cat^T-as-lhsT @ xT-tile           (PE, class logits [106, 128])
    E = exp(FLT)                              (ACT)
    B = E * onehotT                           (DVE, selects label's class row)
    eT = B^T-matmul ones2 -> [128, 2]         (PE, per-sentence exp-logits)
    A2[:, l*64+s] = (iota==seg_rel)*e_l       (DVE tensor_scalar x2)
    u2 += A2^T @ [x | 1]                      (PE, segment-sum accum in PSUM)
- Per chunk epilogue: 1/s scaling, PE transpose, disc projection, +bias.
Numerics: bf16 inputs with fp32 PSUM accumulation everywhere.
"""

import numpy as np

N_SENT = 262144
N_BAGS = 4096
HIDDEN = 256
L0 = 14
NCLS = 53
NCORE = 8
CHUNKS_PER_CORE = 10
NCHUNK = NCORE * CHUNKS_PER_CORE
MAX_BAGS_PER_CHUNK = 64

_CACHE = {}


def _patch_tile_drain():
    # This walrus build rejects Drain instructions carrying more than ~1 sync
    # wait. Split the Tile final-drain waits across SP nops, one wait each.
    import concourse.mybir as mybir
    import concourse.tile as tile_mod
    from concourse.vector_clock import ScopedClock

    if getattr(tile_mod.TileContext, "_drain_split_patched", False):
        return

    def _split_drain_and_barrier(self, tick_clock, wait_clock):
        drain_inst = self.nc.sync.drain()
        wait_clock.add_sem_waits(
            drain_inst.ins, ScopedClock({None: tick_clock.global_clock})
        )
        si = drain_inst.ins.sync_info
        waits = list(si.on_wait) if si is not None else []
        if len(waits) > 1:
            drain_inst.ins.sync_info = mybir.SyncInfo(
                on_wait=waits[:1], on_update=list(si.on_update)
            )
            for w in waits[1:]:
                nop = self.nc.sync.nop(nofuse=True, hint="drain_wait_split")
                nop.ins.sync_info = mybir.SyncInfo(on_wait=[w], on_update=[])
        self.nc.all_engine_barrier()
        assert self.sems is not None
        popped = self.nc._tile_sem_poison_stack.pop()
        assert popped is self._sem_poison
        self.nc.clear_and_free_semaphores(list(self.sems.allocated().values()))
        self.nc.all_engine_barrier()

    tile_mod.TileContext._drain_and_barrier = _split_drain_and_barrier
    tile_mod.TileContext._drain_split_patched = True


def _split_all_waits(nc, max_waits=1):
    """This walrus build caps sync-wait commands per instruction very low.
    Move excess waits onto same-engine NOPs inserted just before."""
    import concourse.mybir as mybir

    n = 0
    for f in nc.m.functions:
        for b in f.blocks:
            new = []
            for inst in b.instructions:
                si = getattr(inst, "sync_info", None)
                waits = list(si.on_wait) if si is not None else []
                if len(waits) > max_waits:
                    keep = waits[: max_waits]
                    extra = waits[max_waits:]
                    for w in extra:
                        nop = mybir.InstNoOp(
                            name=f"waitsplit-{n}", ins=[], outs=[]
                        )
                        n += 1
                        nop.engine = inst.engine
                        nop.sync_info = mybir.SyncInfo(
                            on_wait=[w], on_update=[]
                        )
                        new.append(nop)
                    inst.sync_info = mybir.SyncInfo(
                        on_wait=keep, on_update=list(si.on_update)
                    )
                new.append(inst)
            b.instructions[:] = new
    return n


def _segment_ids(scope):
    marks = np.zeros(N_SENT, np.int64)
    np.add.at(marks, scope[1:-1].astype(np.int64), 1)
    return np.cumsum(marks)


def _balanced_chunks(counts):
    """Partition bags into <=NCHUNK contiguous chunks, <=64 bags each,
    minimizing the max sentence count per chunk. Returns list of (b0, b1)."""
    total = int(counts.sum())

    def greedy(cap):
        bounds = []
        s = 0
        n = 0
        b0 = 0
        for b in range(N_BAGS):
            c = int(counts[b])
            if n == MAX_BAGS_PER_CHUNK or (s + c > cap and n > 0):
                bounds.append((b0, b))
                b0 = b
                s = 0
                n = 0
            s += c
            n += 1
        bounds.append((b0, N_BAGS))
        return bounds

    lo = max(int(counts.max()), total // NCHUNK)
    hi = total
    while lo < hi:
        mid = (lo + hi) // 2
        if len(greedy(mid)) <= NCHUNK:
            hi = mid
        else:
            lo = mid + 1
    bounds = greedy(lo)
    while len(bounds) < NCHUNK:
        bounds.append((N_BAGS, N_BAGS))
    return bounds


def _build_bass(Tc):
    import concourse.mybir as mybir
    from concourse import bass
    from concourse.tile import TileContext

    _patch_tile_drain()
    f32 = mybir.dt.float32
    bf16 = mybir.dt.bfloat16
    AO = mybir.AluOpType
    Narr = CHUNKS_PER_CORE * Tc * 128

    nc = bass.Bass("TRN2")
    d_xh = nc.dram_tensor("xh", [Narr, 257], bf16, kind="ExternalInput")
    d_xt = nc.dram_tensor("xt", [256, Narr], f32, kind="ExternalInput")
    d_oh = nc.dram_tensor("oh", [106, Narr], bf16, kind="ExternalInput")
    d_sg = nc.dram_tensor("segr", [Narr, 1], f32, kind="ExternalInput")
    d_ct = nc.dram_tensor("ct", [2, 128, 106], f32, kind="ExternalInput")
    d_o2 = nc.dram_tensor("ones2", [128, 2], f32, kind="ExternalInput")
    d_io = nc.dram_tensor("iota64", [128, 64], f32, kind="ExternalInput")
    d_id = nc.dram_tensor("ident", [128, 128], f32, kind="ExternalInput")
    d_dt = nc.dram_tensor("dt4", [128, 212], f32, kind="ExternalInput")
    d_bb = nc.dram_tensor("biasb", [64, 53], f32, kind="ExternalInput")
    d_out = nc.dram_tensor(
        "out", [CHUNKS_PER_CORE, 64, 53], f32, kind="ExternalOutput"
    )

    with TileContext(nc) as tc:
        with (
            tc.tile_pool(name="const", bufs=1) as cpool,
            tc.tile_pool(name="xtp", bufs=4) as xtp,
            tc.tile_pool(name="xhp", bufs=4) as xhp,
            tc.tile_pool(name="ohp", bufs=4) as ohp,
            tc.tile_pool(name="sgp", bufs=4) as sgp,
            tc.tile_pool(name="ep", bufs=3) as epool,
            tc.tile_pool(name="a2p", bufs=3) as a2pool,
            tc.tile_pool(name="bp", bufs=1) as bpool,
            tc.tile_pool(name="miscp", bufs=2) as miscp,
            tc.tile_pool(name="ps_flt", bufs=2, space="PSUM") as ps_flt,
            tc.tile_pool(name="ps_et", bufs=2, space="PSUM") as ps_et,
            tc.tile_pool(name="ps_u", bufs=2, space="PSUM") as ps_u,
            tc.tile_pool(name="ps_tp", bufs=1, space="PSUM") as ps_tp,
            tc.tile_pool(name="ps_o", bufs=1, space="PSUM") as ps_o,
        ):
            ct0 = cpool.tile([128, 106], f32, tag="ct0")
            ct1 = cpool.tile([128, 106], f32, tag="ct1")
            o2 = cpool.tile([128, 2], f32, tag="o2")
            io64 = cpool.tile([128, 64], f32, tag="io64")
            ident = cpool.tile([128, 128], f32, tag="ident")
            dt4 = cpool.tile([128, 212], f32, tag="dt4")
            bb = cpool.tile([64, 53], f32, tag="bb")
            nc.sync.dma_start(out=ct0[:], in_=d_ct[0])
            nc.sync.dma_start(out=ct1[:], in_=d_ct[1])
            nc.sync.dma_start(out=o2[:], in_=d_o2[:])
            nc.sync.dma_start(out=io64[:], in_=d_io[:])
            nc.sync.dma_start(out=ident[:], in_=d_id[:])
            nc.sync.dma_start(out=dt4[:], in_=d_dt[:])
            nc.sync.dma_start(out=bb[:], in_=d_bb[:])

            # 3 rotating B buffers with pad rows pre-zeroed (keeps the
            # extraction lhsT at 128 partitions for fast weight load).
            bbufs = [bpool.tile([128, 128], f32, tag=f"b{i}", name=f"b{i}") for i in range(3)]
            for t in bbufs:
                nc.vector.memset(t[96:128, :], 0.0)

            for k in range(CHUNKS_PER_CORE):
                u2 = ps_u.tile([128, 257], f32, tag="u2")
                for t in range(Tc):
                    g = k * Tc + t
                    c0 = g * 128
                    xt0 = xtp.tile([128, 128], f32, tag="xt0")
                    xt1 = xtp.tile([128, 128], f32, tag="xt1")
                    xh = xhp.tile([128, 257], bf16, tag="xh")
                    oht = ohp.tile([106, 128], bf16, tag="oh")
                    sg = sgp.tile([128, 1], f32, tag="sg")
                    nc.sync.dma_start(out=xt0[:], in_=d_xt[0:128, c0 : c0 + 128])
                    nc.sync.dma_start(out=xt1[:], in_=d_xt[128:256, c0 : c0 + 128])
                    nc.sync.dma_start(out=xh[:], in_=d_xh[c0 : c0 + 128, :])
                    nc.sync.dma_start(out=oht[:], in_=d_oh[:, c0 : c0 + 128])
                    nc.sync.dma_start(out=sg[:], in_=d_sg[c0 : c0 + 128, :])

                    flt = ps_flt.tile([106, 128], f32, tag="flt")
                    nc.tensor.matmul(
                        flt[:], ct0[:, 0:106], xt0[:], start=True, stop=False
                    )
                    nc.tensor.matmul(
                        flt[:], ct1[:, 0:106], xt1[:], start=False, stop=True
                    )
                    E = epool.tile([106, 128], f32, tag="E")
                    nc.scalar.activation(
                        E[:], flt[:], mybir.ActivationFunctionType.Exp
                    )
                    Bt = bbufs[g % 3]
                    nc.vector.tensor_tensor(
                        Bt[0:106, :], E[:], oht[:], AO.mult
                    )
                    et = ps_et.tile([128, 2], f32, tag="et")
                    nc.tensor.matmul(et[:], Bt[:], o2[:], start=True, stop=True)

                    a2 = a2pool.tile([128, 128], bf16, tag="a2")
                    nc.vector.tensor_scalar(
                        a2[:, 0:64], io64[:], sg[:], et[:, 0:1],
                        AO.is_equal, AO.mult,
                    )
                    nc.vector.tensor_scalar(
                        a2[:, 64:128], io64[:], sg[:], et[:, 1:2],
                        AO.is_equal, AO.mult,
                    )
                    nc.tensor.matmul(
                        u2[:], a2[:], xh[:],
                        start=(t == 0), stop=(t == Tc - 1),
                    )

                # chunk epilogue
                seps = miscp.tile([128, 1], f32, tag="seps")
                invs = miscp.tile([128, 1], f32, tag="invs")
                nc.vector.tensor_scalar(
                    seps[:], u2[:, 256:257], 1e-30, None, AO.add
                )
                nc.vector.reciprocal(invs[:], seps[:])
                repre = miscp.tile([128, 256], f32, tag="repre")
                nc.vector.tensor_scalar(
                    repre[:], u2[:, 0:256], invs[:], None, AO.mult
                )
                outp = ps_o.tile([64, 53], f32, tag="outp")
                for hh in range(2):
                    tp = ps_tp.tile([128, 128], f32, tag="tp")
                    nc.tensor.transpose(
                        tp[:], repre[:, hh * 128 : (hh + 1) * 128], ident[:]
                    )
                    rT = miscp.tile([128, 128], f32, tag=f"rT{hh}")
                    nc.scalar.copy(rT[:], tp[:])
                    for l in range(2):
                        nc.tensor.matmul(
                            outp[:],
                            rT[:, l * 64 : (l + 1) * 64],
                            dt4[:, (2 * l + hh) * 53 : (2 * l + hh + 1) * 53],
                            start=(hh == 0 and l == 0),
                            stop=(hh == 1 and l == 1),
                        )
                outs = miscp.tile([64, 53], f32, tag="outs")
                nc.vector.tensor_tensor(outs[:], outp[:], bb[:], AO.add)
                nc.sync.dma_start(out=d_out[k], in_=outs[:])

    _split_all_waits(nc)
    return nc


def _prep(x, rel_emb0, rel_emb1, disc, bias, relation_levels, label_index, scope):
    import concourse.mybir as mybir

    bf = mybir.dt.np(mybir.dt.bfloat16)
    seg = _segment_ids(np.asarray(scope))
    counts = np.bincount(seg, minlength=N_BAGS).astype(np.int64)
    cum = np.concatenate([[0], np.cumsum(counts)])
    bounds = _balanced_chunks(counts)
    max_sents = max(int(cum[b1] - cum[b0]) for b0, b1 in bounds)
    Tc = max(1, (max_sents + 127) // 128)
    Narr = CHUNKS_PER_CORE * Tc * 128

    x = np.asarray(x, np.float32)
    labels = np.asarray(label_index, np.int64)
    xbf = x.astype(bf)
    xT32 = np.ascontiguousarray(x.T)

    rl = np.asarray(relation_levels, np.int64)
    c0 = np.asarray(rel_emb0, np.float32)[rl[:, 0]]
    c1 = np.asarray(rel_emb1, np.float32)[rl[:, 1]]
    ccat = np.concatenate([c0, c1], 0)  # [106, 256]
    ctT = np.ascontiguousarray(ccat.T)  # [256, 106]
    ct = np.stack([ctT[0:128], ctT[128:256]], 0).astype(np.float32)

    ones2 = np.zeros((128, 2), np.float32)
    ones2[0:53, 0] = 1.0
    ones2[53:106, 1] = 1.0
    iota64 = np.broadcast_to(np.arange(64, dtype=np.float32), (128, 64)).copy()
    ident = np.eye(128, dtype=np.float32)
    disc = np.asarray(disc, np.float32)
    dt4 = np.zeros((2, 2, 128, 53), np.float32)
    for l in range(2):
        for hh in range(2):
            dt4[l, hh] = disc[:, l * 256 + hh * 128 : l * 256 + (hh + 1) * 128].T
    dt4 = np.ascontiguousarray(dt4.transpose(2, 0, 1, 3)).reshape(128, 212)
    biasb = np.broadcast_to(np.asarray(bias, np.float32), (64, 53)).copy()

    const = {
        "ct": ct,
        "ones2": ones2,
        "iota64": iota64,
        "ident": ident,
        "dt4": dt4,
        "biasb": biasb,
    }

    in_maps = []
    meta = []
    ohg = (labels[None, :] == np.arange(53)[:, None])
    for core in range(NCORE):
        xh = np.zeros((Narr, 257), bf)
        xt = np.zeros((256, Narr), np.float32)
        oh = np.zeros((106, Narr), bf)
        sg = np.full((Narr, 1), -1.0, np.float32)
        cmeta = []
        for k in range(CHUNKS_PER_CORE):
            b0, b1 = bounds[core * CHUNKS_PER_CORE + k]
            s0, s1 = int(cum[b0]), int(cum[b1])
            L = s1 - s0
            off = k * Tc * 128
            if L > 0:
                xh[off : off + L, 0:256] = xbf[s0:s1]
                xh[off : off + L, 256] = 1.0
                xt[:, off : off + L] = xT32[:, s0:s1]
                o = ohg[:, s0:s1]
                oh[0:53, off : off + L] = o
                oh[53:106, off : off + L] = o
                sg[off : off + L, 0] = (seg[s0:s1] - b0).astype(np.float32)
            cmeta.append((b0, b1))
        meta.append(cmeta)
        in_maps.append(
            {"xh": xh, "xt": xt, "oh": oh, "segr": sg, **const}
        )
    return Tc, in_maps, meta


def kernel(x, rel_emb0, rel_emb1, disc, bias, relation_levels, label_index,
           scope, _trace=False):
    from concourse.bass_utils import run_bass_kernel_spmd

    Tc, in_maps, meta = _prep(
        x, rel_emb0, rel_emb1, disc, bias, relation_levels, label_index, scope
    )
    if Tc not in _CACHE:
        _CACHE[Tc] = _build_bass(Tc)
    nc = _CACHE[Tc]
    res = None
    for attempt in range(3):
        try:
            res = run_bass_kernel_spmd(
                nc, in_maps, core_ids=list(range(NCORE)), trace=_trace
            )
            break
        except Exception:
            if attempt == 2:
                raise
    out = np.zeros((N_BAGS, NCLS), np.float32)
    for core in range(NCORE):
        o = np.asarray(res.results[core]["out"])
        for k, (b0, b1) in enumerate(meta[core]):
            if b1 > b0:
                out[b0:b1] = o[k, : b1 - b0]
    kernel._last_results = res
    return out

